# revision 1
# baseline (speedup 1.0000x reference)
"""Bidirectional 2-layer GRU (BS=32, T=2048, D=H=256) on 8 trn2 NeuronCores.

Time-parallel recurrence with warmup. The GRU update is contractive
(z-gate forgetting), so a segment started from h=0 at t0-W converges to the
true trajectory by t0 (W=12 -> max abs err ~2e-3, fp64-verified).

Sharding: core c = (layer l = c//4, k = c%4). Per direction the 2048 steps
split into P=16 windows of 128 steps; core k owns windows {4k..4k+3} of both
directions for its layer: NG=4 groups x (2 windows x 32 batch) = 64 lanes
each. All groups advance one step per "round"; R = 128 + W rounds total.
Groups are independent chains; their elementwise/matmul instructions
interleave so every engine stays busy (4 chains hide the ~3.5us per-step
dependency latency).

Per round per group (gates-on-partitions, 6 gate tiles of 128):
  PSUM tile ps[128, 8, 64] (1 bank): regions rz=0:4, n=4:6, xn=6:8.
  PE: 1 K=8 bias matmul (indicator trick seeds all biases), 8 Wx_rz + 4 Wx_n
      matmuls straight off x_t (no gx precompute), then 12 Wh matmuls (the
      only h-dependent part, emitted last).
  ACT: sigmoid(ps[0:4]) -> r,z;  tanh(an) -> n.
  DVE/Pool: rn = ps_n*r; an = ps_xn + rn; zh = z*h; zn = (z-1)*n;
            h' = zh - zn  (= (1-z)n + z*h), written into the output stage.
State/weights fp16; x DMA'd per C-round chunk in consumption order (bwd
streams pre-reversed on host); boundary streams (fwd w0, bwd w15) get their
post-warmup state zeroed by a mask multiply at round W-1.
"""

import os
from contextlib import ExitStack

import numpy as np

import concourse.bass as bass
from concourse import mybir
from concourse.alu_op_type import AluOpType
from concourse.tile import TileContext
from concourse.bass_utils import run_bass_kernel_spmd

BS, T_FULL, D = 32, 2048, 256
H, L = 256, 2
NG = 4          # groups per core
LAN = 64        # lanes per group (2 windows x 32 batch)
NW = 2          # windows per group
PW = 16         # windows per direction
SEG = T_FULL // PW  # 128 steps per window
W = 12          # warmup rounds
R = SEG + W     # rounds
C = 4           # rounds per DMA chunk
NCH = R // C

F16 = mybir.dt.float16
F32 = mybir.dt.float32
AF = mybir.ActivationFunctionType


def _fix_drain_waits(nc, max_waits=1):
    """This container's walrus rejects instructions carrying more than one
    sync-wait. Tile may attach several. Split: keep the last wait on the
    instruction and hoist the others onto single-wait NOPs placed just before
    it on the same engine (engine streams are serial, so semantics match)."""
    n_new = 0
    for f in nc.m.functions:
        for bb in f.blocks:
            insts = list(bb.instructions)
            out = []
            changed = False
            for inst in insts:
                si = inst.sync_info
                if si and len(si.on_wait) > max_waits:
                    waits = list(si.on_wait)
                    for k, w in enumerate(waits[:-max_waits]):
                        nd = mybir.InstNoOp(name=f"{inst.name}-w{k}", ins=[], outs=[])
                        nd.engine = inst.engine
                        nd.sync_info = mybir.SyncInfo(on_wait=[w], on_update=[])
                        out.append(nd)
                        nc.register_instruction(nd, overwrite=True)
                        n_new += 1
                    inst.sync_info = mybir.SyncInfo(
                        on_wait=waits[-max_waits:], on_update=list(si.on_update)
                    )
                    changed = True
                out.append(inst)
            if changed:
                lst = bb.instructions
                lst.clear()
                lst.extend(out)
                assert [i.name for i in bb.instructions] == [i.name for i in out]
    return n_new


def _build():
    nc = bass.Bass(name="bidir_gru_tp3", trn_type="TRN2")

    # x in consumption order: [ch, p, kcx, j, g, lane]
    xt = nc.dram_tensor("xt", [NCH, 128, 2, C, NG, LAN], F16, kind="ExternalInput")
    wht = nc.dram_tensor("wht", [128, 12, 128], F16, kind="ExternalInput")
    wxt = nc.dram_tensor("wxt", [128, 12, 128], F16, kind="ExternalInput")
    # K=8 bias stationary: rows 0:4 brz (bx+bh), 4:6 bhn, 6:8 bxn
    bias8 = nc.dram_tensor("bias8", [8, 128], F16, kind="ExternalInput")
    hmask = nc.dram_tensor("hmask", [128, 2, NG, LAN], F16, kind="ExternalInput")
    ind8d = nc.dram_tensor("ind8d", [8, 8, LAN], F16, kind="ExternalInput")
    # out[ch, p, j, kc, g, lane]
    out = nc.dram_tensor("out", [NCH, 128, C, 2, NG, LAN], F16, kind="ExternalOutput")

    with TileContext(nc) as tc, ExitStack() as ctx:
        const = ctx.enter_context(tc.tile_pool(name="const", bufs=1))
        xtp = ctx.enter_context(tc.tile_pool(name="xtp", bufs=2))
        outp = ctx.enter_context(tc.tile_pool(name="outp", bufs=2))
        psp = ctx.enter_context(tc.tile_pool(name="psp", bufs=2, space="PSUM"))
        ew = ctx.enter_context(tc.tile_pool(name="ew", bufs=3))

        wht_sb = const.tile([128, 12, 128], F16)
        nc.sync.dma_start(out=wht_sb, in_=wht[:, :, :])
        wxt_sb = const.tile([128, 12, 128], F16)
        nc.sync.dma_start(out=wxt_sb, in_=wxt[:, :, :])
        bias8_sb = const.tile([8, 128], F16)
        nc.sync.dma_start(out=bias8_sb, in_=bias8[:, :])
        hmask_sb = const.tile([128, 2, NG, LAN], F16)
        nc.sync.dma_start(out=hmask_sb, in_=hmask[:, :, :, :])
        # indicator for the K=8 bias matmul: ind8[k, reg, lane] = (k == reg)
        ind8 = const.tile([8, 8, LAN], F16)
        nc.sync.dma_start(out=ind8, in_=ind8d[:, :, :])
        zeros = const.tile([128, 2, NG, LAN], F16)
        nc.vector.memset(zeros, 0.0)

        # per-group h state AP (points into outc tiles after round 0)
        h_prev = [zeros[:, :, g, :] for g in range(NG)]

        outc = None
        for ch in range(NCH):
            xt_sb = xtp.tile([128, 2, C, NG, LAN], F16, tag="xt")
            nc.sync.dma_start(out=xt_sb[:, 0, :, :, :], in_=xt[ch, :, 0])
            nc.sync.dma_start(out=xt_sb[:, 1, :, :, :], in_=xt[ch, :, 1])
            outc_prev = outc
            outc = outp.tile([128, C, 2, NG, LAN], F16, tag="outc")
            for j in range(C):
                r = ch * C + j
                ps_all = []
                for g in range(NG):
                    # ---- PE: h-independent part first. ONE start=True per
                    # PSUM bank (the K=8 bias seed covers the whole tile) ----
                    ps = psp.tile([128, 8, LAN], F32, tag=f"ps{g}")
                    ps_all.append(ps)
                    nc.tensor.matmul(
                        out=ps.rearrange("p a b -> p (a b)"),
                        lhsT=bias8_sb[:, :],
                        rhs=ind8.rearrange("k a b -> k (a b)"),
                        start=True,
                        stop=False,
                    )
                    for mt in range(4):  # Wx for r,z -> ps[0:4]
                        for kx in range(2):
                            nc.tensor.matmul(
                                out=ps[:, mt, :],
                                lhsT=wxt_sb[:, kx * 6 + mt, :],
                                rhs=xt_sb[:, kx, j, g, :],
                                start=False,
                                stop=False,
                            )
                    for mt in range(4, 6):  # Wx_n -> ps[6:8]
                        for kx in range(2):
                            nc.tensor.matmul(
                                out=ps[:, mt + 2, :],
                                lhsT=wxt_sb[:, kx * 6 + mt, :],
                                rhs=xt_sb[:, kx, j, g, :],
                                start=False,
                                stop=(r == 0 and mt == 5 and kx == 1),
                            )
                for g in range(NG):
                    # ---- PE: recurrent matmuls (wait on h'(r-1)); at round 0
                    # the state is exactly zero, so Wh contributes nothing ----
                    if r == 0:
                        continue
                    ps = ps_all[g]
                    hp = h_prev[g]
                    for mt in range(6):
                        for kc in range(2):
                            nc.tensor.matmul(
                                out=ps[:, mt, :],
                                lhsT=wht_sb[:, kc * 6 + mt, :],
                                rhs=hp[:, kc, :],
                                start=False,
                                stop=(mt == 5 and kc == 1),
                            )
                sgl = [None] * NG
                anl = [None] * NG
                ntl = [None] * NG
                zhl = [None] * NG
                zml = [None] * NG

                def em_sig(g):
                    sg = ew.tile([128, 4, LAN], F16, tag=f"sg{g}")
                    nc.scalar.activation(
                        out=sg, in_=ps_all[g][:, 0:4, :], func=AF.Sigmoid
                    )
                    sgl[g] = sg

                def em_mid(g):
                    rn = ew.tile([128, 2, LAN], F16, tag=f"rn{g}")
                    nc.vector.tensor_tensor(
                        out=rn, in0=ps_all[g][:, 4:6, :], in1=sgl[g][:, 0:2, :],
                        op=AluOpType.mult,
                    )
                    an = ew.tile([128, 2, LAN], F16, tag=f"an{g}")
                    nc.vector.tensor_tensor(
                        out=an, in0=ps_all[g][:, 6:8, :], in1=rn, op=AluOpType.add
                    )
                    anl[g] = an
                    zh = ew.tile([128, 2, LAN], F16, tag=f"zh{g}")
                    nc.gpsimd.tensor_tensor(
                        out=zh, in0=sgl[g][:, 2:4, :], in1=h_prev[g],
                        op=AluOpType.mult,
                    )
                    zhl[g] = zh
                    zm1 = ew.tile([128, 2, LAN], F16, tag=f"zm1{g}")
                    nc.gpsimd.tensor_scalar(
                        out=zm1, in0=sgl[g][:, 2:4, :], scalar1=-1.0, scalar2=None,
                        op0=AluOpType.add,
                    )
                    zml[g] = zm1

                def em_tanh(g):
                    nt = ew.tile([128, 2, LAN], F16, tag=f"nt{g}")
                    nc.scalar.activation(out=nt, in_=anl[g], func=AF.Tanh)
                    ntl[g] = nt

                def em_tail(g):
                    zn = ew.tile([128, 2, LAN], F16, tag=f"zn{g}")
                    nc.vector.tensor_tensor(
                        out=zn, in0=zml[g], in1=ntl[g], op=AluOpType.mult
                    )
                    dst = outc[:, j, :, g, :]
                    nc.vector.tensor_sub(dst, zhl[g], zn)
                    h_prev[g] = dst

                em_sig(0)
                em_mid(0)
                em_sig(1)
                em_tanh(0)
                em_mid(1)
                em_sig(2)
                em_tanh(1)
                em_tail(0)
                em_mid(2)
                em_sig(3)
                em_tanh(2)
                em_tail(1)
                em_mid(3)
                em_tanh(3)
                em_tail(2)
                em_tail(3)
                if r == W - 1:
                    # zero post-warmup state of boundary streams (fwd w0 on
                    # core0, bwd w15 on core3); identity elsewhere. Per-group
                    # ops so the groups' chains stay unsynchronized.
                    for g in range(NG):
                        hm = outc[:, j, :, g, :]
                        nc.vector.tensor_tensor(
                            out=hm, in0=hm, in1=hmask_sb[:, :, g, :],
                            op=AluOpType.mult,
                        )
                        h_prev[g] = hm
            nc.sync.dma_start(out=out[ch], in_=outc)
            del outc_prev

    _fix_drain_waits(nc)
    return nc


_CACHE = {}


def _get_nc(T=T_FULL):
    assert T == T_FULL, "v2 kernel is specialized to T=2048"
    if T not in _CACHE:
        _CACHE[T] = _build()
    return _CACHE[T]


def _window_tidx():
    """t index per (group, wsub, round) for one core's layer-slice.
    Returns int array [NG, NW, R] given core index k (set later)."""
    r = np.arange(R)
    return r


def prep_in_maps(x, Wx, Wh, bx, bh):
    x = np.asarray(x, np.float32)
    Wx = np.asarray(Wx, np.float32)
    Wh = np.asarray(Wh, np.float32)
    bx = np.asarray(bx, np.float32)
    bh = np.asarray(bh, np.float32)
    T = x.shape[1]
    assert T == T_FULL

    rr = np.arange(R)
    in_maps = []
    for c in range(8):
        l, k = c // 4, c % 4
        # windows per group: g0: fwd {4k, 4k+1}; g1: fwd {4k+2, 4k+3};
        #                    g2: bwd {4k, 4k+1}; g3: bwd {4k+2, 4k+3}
        tidx = np.empty((NG, NW, R), np.int64)
        for g in range(NG):
            fwd = g < 2
            for ws in range(NW):
                w = 4 * k + (g % 2) * 2 + ws
                if fwd:
                    t = SEG * w - W + rr
                else:
                    t = SEG * (w + 1) - 1 + W - rr
                tidx[g, ws] = np.clip(t, 0, T - 1)
        # gather: [b, g, ws, r, d]
        xg = x[:, tidx, :].astype(np.float16)  # (32, NG, NW, R, 256)
        # -> [ch, p, kcx, j, g, (ws, b)]
        xg = xg.reshape(BS, NG, NW, NCH, C, 2, 128)
        xt_h = np.ascontiguousarray(xg.transpose(3, 6, 5, 4, 1, 2, 0)).reshape(
            NCH, 128, 2, C, NG, LAN
        )

        wht_h = np.ascontiguousarray(
            Wh[l].reshape(6, 128, 2, 128).transpose(3, 2, 0, 1).reshape(128, 12, 128),
            np.float16,
        )
        wxt_h = np.ascontiguousarray(
            Wx[l].reshape(6, 128, 2, 128).transpose(3, 2, 0, 1).reshape(128, 12, 128),
            np.float16,
        )
        bsum = bx[l] + bh[l]
        b8 = np.empty((8, 128), np.float32)
        b8[0:4] = bsum[0:512].reshape(4, 128)
        b8[4:6] = bh[l][512:768].reshape(2, 128)
        b8[6:8] = bx[l][512:768].reshape(2, 128)

        ind8_h = np.zeros((8, 8, LAN), np.float16)
        for kk in range(8):
            ind8_h[kk, kk, :] = 1.0
        hm = np.ones((128, 2, NG, LAN), np.float16)
        if k == 0:
            hm[:, :, 0, 0:32] = 0.0  # fwd window 0
        if k == 3:
            hm[:, :, 3, 32:64] = 0.0  # bwd window 15
        in_maps.append(
            {
                "xt": xt_h,
                "wht": wht_h,
                "wxt": wxt_h,
                "bias8": b8.astype(np.float16),
                "hmask": hm,
                "ind8d": ind8_h,
            }
        )
    return in_maps


def assemble_out(per_core_out, T=T_FULL):
    OUT = np.empty((BS, T * L, 2 * H), np.float32)
    for c in range(8):
        l, k = c // 4, c % 4
        o = np.asarray(per_core_out[c], np.float32).reshape(NCH, 128, C, 2, NG, LAN)
        # -> [r, kc, p, g, ws, b]
        o = o.transpose(0, 2, 3, 1, 4, 5).reshape(R, 2, 128, NG, NW, BS)
        o = o.reshape(R, 256, NG, NW, BS)
        kept = o[W : W + SEG]  # [seg_j, 256, NG, NW, b]
        for g in range(NG):
            fwd = g < 2
            for ws in range(NW):
                w = 4 * k + (g % 2) * 2 + ws
                hs = kept[:, :, g, ws, :]  # [seg_j, 256, b]
                if not fwd:
                    hs = hs[::-1]
                ts = np.arange(SEG * w, SEG * (w + 1))
                col0 = 0 if fwd else 256
                OUT[:, 2 * ts + l, col0 : col0 + 256] = hs.transpose(2, 0, 1)
    return OUT


def kernel(x, Wx, Wh, bx, bh):
    T = x.shape[1]
    nc = _get_nc(T)
    in_maps = prep_in_maps(x, Wx, Wh, bx, bh)
    res = run_bass_kernel_spmd(nc, in_maps, core_ids=list(range(8)))
    kernel.last_results = res
    return assemble_out([r["out"] for r in res.results], T)



# revision 4
# speedup vs baseline: 1.0396x; 1.0396x over previous
"""Bidirectional 2-layer GRU (BS=32, T=2048, D=H=256) on 8 trn2 NeuronCores.

v2: P=32 windows of SEG=64 steps (+W=12 warmup), NG=4 chains x LAN=128 lanes
(4 windows x 32 batch). Bigger tiles amortize fixed instruction overheads vs
the v1 LAN=64 design; engines balanced:
  PE   : rz-bias seed (K=4 indicator matmul) + 12 Wx + 12 Wh per group-round.
         z-gate rows of Wx/Wh (and bias) are negated on host so sigmoid
         yields z' = 1-z directly.
  DVE  : rn = (ps_hn + bhn)*r and an = (ps_xn + bxn) + rn as
         scalar_tensor_tensor pairs (per-partition bias APs fold the n-side
         biases for free); d = n - h.
  ACT  : sigmoid(ps_rz) -> [r, z'], tanh(an) -> n.
  POOL : e = d*z', h' = h + e (scalar_tensor_tensor forms, 0.6 eff).
PSUM: 4 tiles of [128, 8, 128] fp32 (2 banks each) = all 8 banks, bufs=1.
Regions: 0:4 rz (seeded with biases), 4:6 hn, 6:8 xn.
"""

import os
from contextlib import ExitStack

import numpy as np

import concourse.bass as bass
from concourse import mybir
from concourse.alu_op_type import AluOpType
from concourse.tile import TileContext
from concourse.bass_utils import run_bass_kernel_spmd

BS, T_FULL, D = 32, 2048, 256
H, L = 256, 2
NG = 4            # chains per core
LAN = 128         # lanes per chain (4 windows x 32 batch)
NWIN = 4          # windows per chain
PW = 32           # windows per direction
SEG = T_FULL // PW  # 64 steps per window
W = 12            # warmup rounds
R = SEG + W       # 76 rounds
C = 4             # rounds per DMA chunk
NCH = R // C      # 19

F16 = mybir.dt.float16
F32 = mybir.dt.float32
AF = mybir.ActivationFunctionType
AL = AluOpType


def _fix_drain_waits(nc, max_waits=1):
    """Walrus rejects instructions with >1 sync-wait: split extras onto
    single-wait NOPs just before, on the same engine."""
    n_new = 0
    for f in nc.m.functions:
        for bb in f.blocks:
            insts = list(bb.instructions)
            out = []
            changed = False
            for inst in insts:
                si = inst.sync_info
                if si and len(si.on_wait) > max_waits:
                    waits = list(si.on_wait)
                    for k, w in enumerate(waits[:-max_waits]):
                        nd = mybir.InstNoOp(name=f"{inst.name}-w{k}", ins=[], outs=[])
                        nd.engine = inst.engine
                        nd.sync_info = mybir.SyncInfo(on_wait=[w], on_update=[])
                        out.append(nd)
                        nc.register_instruction(nd, overwrite=True)
                        n_new += 1
                    inst.sync_info = mybir.SyncInfo(
                        on_wait=waits[-max_waits:], on_update=list(si.on_update)
                    )
                    changed = True
                out.append(inst)
            if changed:
                lst = bb.instructions
                lst.clear()
                lst.extend(out)
                assert [i.name for i in bb.instructions] == [i.name for i in out]
    return n_new


def _build():
    nc = bass.Bass(name="bidir_gru_v2", trn_type="TRN2")

    # x in consumption order: [ch, p, kx, j, g, lane]
    xt = nc.dram_tensor("xt", [NCH, 128, 2, C, NG, LAN], F16, kind="ExternalInput")
    # weights [p(K-half), kc*6+mt, gate-col] with z-rows negated
    wht = nc.dram_tensor("wht", [128, 12, 128], F16, kind="ExternalInput")
    wxt = nc.dram_tensor("wxt", [128, 12, 128], F16, kind="ExternalInput")
    # K=4 rz-bias seed: b4[reg, p] = bias value of gate-tile reg, col p
    b4d = nc.dram_tensor("b4d", [4, 128], F16, kind="ExternalInput")
    ind4d = nc.dram_tensor("ind4d", [4, 4, LAN], F16, kind="ExternalInput")
    # n-side biases as per-partition scalar columns
    bhn2d = nc.dram_tensor("bhn2d", [128, 2], F32, kind="ExternalInput")
    bxn2d = nc.dram_tensor("bxn2d", [128, 2], F32, kind="ExternalInput")
    hmask = nc.dram_tensor("hmask", [128, 2, NG, LAN], F16, kind="ExternalInput")
    # out[ch, p, j, kc, g, lane]
    out = nc.dram_tensor("out", [NCH, 128, C, 2, NG, LAN], F16, kind="ExternalOutput")

    with TileContext(nc) as tc, ExitStack() as ctx:
        const = ctx.enter_context(tc.tile_pool(name="const", bufs=1))
        xtp = ctx.enter_context(tc.tile_pool(name="xtp", bufs=2))
        outp = ctx.enter_context(tc.tile_pool(name="outp", bufs=2))
        psp = ctx.enter_context(tc.tile_pool(name="psp", bufs=1, space="PSUM"))
        ew = ctx.enter_context(tc.tile_pool(name="ew", bufs=3))

        wht_sb = const.tile([128, 12, 128], F16)
        nc.sync.dma_start(out=wht_sb, in_=wht[:, :, :])
        wxt_sb = const.tile([128, 12, 128], F16)
        nc.sync.dma_start(out=wxt_sb, in_=wxt[:, :, :])
        b4_sb = const.tile([4, 128], F16)
        nc.sync.dma_start(out=b4_sb, in_=b4d[:, :])
        ind4 = const.tile([4, 4, LAN], F16)
        nc.sync.dma_start(out=ind4, in_=ind4d[:, :, :])
        bhn2 = const.tile([128, 2], F32)
        nc.sync.dma_start(out=bhn2, in_=bhn2d[:, :])
        bxn2 = const.tile([128, 2], F32)
        nc.sync.dma_start(out=bxn2, in_=bxn2d[:, :])
        hmask_sb = const.tile([128, 2, NG, LAN], F16)
        nc.sync.dma_start(out=hmask_sb, in_=hmask[:, :, :, :])
        zeros = const.tile([128, 2, NG, LAN], F16)
        nc.vector.memset(zeros, 0.0)

        h_prev = [zeros[:, :, g, :] for g in range(NG)]

        outc = None
        for ch in range(NCH):
            xt_sb = xtp.tile([128, 2, C, NG, LAN], F16, tag="xt")
            nc.sync.dma_start(out=xt_sb, in_=xt[ch])
            outc_prev = outc
            outc = outp.tile([128, C, 2, NG, LAN], F16, tag="outc")
            for j in range(C):
                r = ch * C + j
                rz_all = []
                nn_all = []
                for g in range(NG):
                    # --- PE h-independent phase: rz-bias seed + Wx ---
                    # rz tile: regions 0:4 = [r0, r1, z'0, z'1] (1 PSUM bank)
                    # nn tile: regions 0:2 = hn, 2:4 = xn (1 PSUM bank)
                    rz = psp.tile([128, 4, LAN], F32, tag=f"rz{g}")
                    nn = psp.tile([128, 4, LAN], F32, tag=f"nn{g}")
                    rz_all.append(rz)
                    nn_all.append(nn)
                    # seed rz with biases (z rows negated)
                    nc.tensor.matmul(
                        out=rz.rearrange("p a b -> p (a b)"),
                        lhsT=b4_sb[:, :],
                        rhs=ind4.rearrange("k a b -> k (a b)"),
                        start=True,
                        stop=False,
                    )
                    for mt in range(4):  # Wx_rz
                        for kx in range(2):
                            nc.tensor.matmul(
                                out=rz[:, mt, :],
                                lhsT=wxt_sb[:, kx * 6 + mt, :],
                                rhs=xt_sb[:, kx, j, g, :],
                                start=False,
                                stop=False,
                            )
                    for mt in range(4, 6):  # Wx_n -> nn regions 2:4
                        for kx in range(2):
                            nc.tensor.matmul(
                                out=nn[:, mt - 2, :],
                                lhsT=wxt_sb[:, kx * 6 + mt, :],
                                rhs=xt_sb[:, kx, j, g, :],
                                start=(kx == 0),
                                stop=False,
                            )
                for g in range(NG):
                    # --- PE recurrent phase: Wh (rz accum, hn fresh) ---
                    rz, nn = rz_all[g], nn_all[g]
                    hp = h_prev[g]
                    for mt in range(6):
                        for kc in range(2):
                            nc.tensor.matmul(
                                out=rz[:, mt, :] if mt < 4 else nn[:, mt - 4, :],
                                lhsT=wht_sb[:, kc * 6 + mt, :],
                                rhs=hp[:, kc, :],
                                start=(mt >= 4 and kc == 0),
                                stop=(mt == 3 and kc == 1)
                                if mt < 4
                                else (mt == 5 and kc == 1),
                            )

                sgl = [None] * NG
                rnl = [None] * NG
                anl = [None] * NG
                ntl = [None] * NG
                dl = [None] * NG

                def em_sig(g):
                    # sg = [r0, r1, z'0, z'1]
                    sg = ew.tile([128, 4, LAN], F16, tag=f"sg{g}")
                    nc.scalar.activation(
                        out=sg, in_=rz_all[g], func=AF.Sigmoid
                    )
                    sgl[g] = sg

                def em_rn(g):
                    # rn = (ps_hn + bhn) * r   (2 stt, per-partition bias)
                    rn = ew.tile([128, 2, LAN], F16, tag=f"rn{g}")
                    for kc in range(2):
                        nc.vector.scalar_tensor_tensor(
                            out=rn[:, kc, :],
                            in0=nn_all[g][:, kc, :],
                            scalar=bhn2[:, kc : kc + 1],
                            in1=sgl[g][:, kc, :],
                            op0=AL.add,
                            op1=AL.mult,
                        )
                    rnl[g] = rn

                def em_an(g):
                    # an = (ps_xn + bxn) + rn  (2 stt)
                    an = ew.tile([128, 2, LAN], F16, tag=f"an{g}")
                    for kc in range(2):
                        nc.vector.scalar_tensor_tensor(
                            out=an[:, kc, :],
                            in0=nn_all[g][:, 2 + kc, :],
                            scalar=bxn2[:, kc : kc + 1],
                            in1=rnl[g][:, kc, :],
                            op0=AL.add,
                            op1=AL.add,
                        )
                    anl[g] = an

                def em_tanh(g):
                    nt = ew.tile([128, 2, LAN], F16, tag=f"nt{g}")
                    nc.scalar.activation(out=nt, in_=anl[g], func=AF.Tanh)
                    ntl[g] = nt

                def em_d(g):
                    # d = n - h ; e = d * z'  (DVE, sbuf 2x, back-to-back)
                    d = ew.tile([128, 2, LAN], F16, tag=f"d{g}")
                    nc.vector.tensor_tensor(
                        out=d, in0=ntl[g], in1=h_prev[g], op=AL.subtract
                    )
                    e = ew.tile([128, 2, LAN], F16, tag=f"e{g}")
                    nc.vector.tensor_tensor(
                        out=e, in0=d, in1=sgl[g][:, 2:4, :], op=AL.mult
                    )
                    dl[g] = e

                def em_tail(g):
                    # h' = h + e  (POOL tensor_tensor -- TensorScalarPtr is
                    # not a legal Pool opcode)
                    dst = outc[:, j, :, g, :]
                    nc.gpsimd.tensor_tensor(
                        out=dst, in0=h_prev[g], in1=dl[g], op=AL.add,
                    )
                    h_prev[g] = dst

                # software-pipelined interleave across the 4 chains; DVE
                # order completes ALL rn/an (they gate next round's PE
                # writes) before the d/e tail ops
                em_sig(0)
                em_rn(0)
                em_an(0)
                em_sig(1)
                em_tanh(0)
                em_rn(1)
                em_an(1)
                em_d(0)
                em_sig(2)
                em_tail(0)
                em_tanh(1)
                em_rn(2)
                em_an(2)
                em_d(1)
                em_sig(3)
                em_tail(1)
                em_tanh(2)
                em_rn(3)
                em_an(3)
                em_d(2)
                em_tail(2)
                em_tanh(3)
                em_d(3)
                em_tail(3)

                if r == W - 1:
                    # zero post-warmup state of boundary streams
                    for g in range(NG):
                        hm = outc[:, j, :, g, :]
                        nc.vector.tensor_tensor(
                            out=hm, in0=hm, in1=hmask_sb[:, :, g, :],
                            op=AL.mult,
                        )
                        h_prev[g] = hm
            nc.sync.dma_start(out=out[ch], in_=outc)
            del outc_prev

    _fix_drain_waits(nc)
    return nc


_CACHE = {}


def _get_nc(T=T_FULL):
    assert T == T_FULL, "v2 kernel is specialized to T=2048"
    if T not in _CACHE:
        _CACHE[T] = _build()
    return _CACHE[T]


def prep_in_maps(x, Wx, Wh, bx, bh):
    x = np.asarray(x, np.float32)
    Wx = np.asarray(Wx, np.float32).copy()
    Wh = np.asarray(Wh, np.float32).copy()
    bx = np.asarray(bx, np.float32)
    bh = np.asarray(bh, np.float32)
    T = x.shape[1]
    assert T == T_FULL

    # negate z-gate rows so sigmoid gives z' = 1-z
    Wx[:, 256:512, :] *= -1.0
    Wh[:, 256:512, :] *= -1.0
    brz = (bx + bh)[:, 0:512].copy()
    brz[:, 256:512] *= -1.0

    rr = np.arange(R)
    in_maps = []
    for c in range(8):
        l, k = c // 4, c % 4
        # chains: g0: fwd w 8k+0..3; g1: fwd w 8k+4..7; g2/g3: bwd same
        tidx = np.empty((NG, NWIN, R), np.int64)
        for g in range(NG):
            fwd = g < 2
            for ws in range(NWIN):
                w = 8 * k + (g % 2) * 4 + ws
                if fwd:
                    t = SEG * w - W + rr
                else:
                    t = SEG * (w + 1) - 1 + W - rr
                tidx[g, ws] = np.clip(t, 0, T - 1)
        # gather: [b, g, ws, r, d] -> [ch, p, kx, j, g, (ws, b)]
        xg = x[:, tidx, :].astype(np.float16)  # (32, NG, NWIN, R, 256)
        xg = xg.reshape(BS, NG, NWIN, NCH, C, 2, 128)
        xt_h = np.ascontiguousarray(xg.transpose(3, 6, 5, 4, 1, 2, 0)).reshape(
            NCH, 128, 2, C, NG, LAN
        )

        wht_h = np.ascontiguousarray(
            Wh[l].reshape(6, 128, 2, 128).transpose(3, 2, 0, 1).reshape(128, 12, 128),
            np.float16,
        )
        wxt_h = np.ascontiguousarray(
            Wx[l].reshape(6, 128, 2, 128).transpose(3, 2, 0, 1).reshape(128, 12, 128),
            np.float16,
        )
        b4 = brz[l].reshape(4, 128).astype(np.float16)
        ind4_h = np.zeros((4, 4, LAN), np.float16)
        for kk in range(4):
            ind4_h[kk, kk, :] = 1.0
        bhn2_h = np.ascontiguousarray(bh[l, 512:768].reshape(2, 128).T, np.float32)
        bxn2_h = np.ascontiguousarray(bx[l, 512:768].reshape(2, 128).T, np.float32)

        hm = np.ones((128, 2, NG, LAN), np.float16)
        if k == 0:
            hm[:, :, 0, 0:32] = 0.0  # fwd window 0
        if k == 3:
            hm[:, :, 3, 96:128] = 0.0  # bwd window 31
        in_maps.append(
            {
                "xt": xt_h,
                "wht": wht_h,
                "wxt": wxt_h,
                "b4d": b4,
                "ind4d": ind4_h,
                "bhn2d": bhn2_h,
                "bxn2d": bxn2_h,
                "hmask": hm,
            }
        )
    return in_maps


def assemble_out(per_core_out, T=T_FULL):
    OUT = np.empty((BS, T * L, 2 * H), np.float32)
    for c in range(8):
        l, k = c // 4, c % 4
        o = np.asarray(per_core_out[c], np.float32).reshape(NCH, 128, C, 2, NG, LAN)
        # [ch, p, j, kc, g, lane] -> [r, kc, p, g, ws, b]
        o = o.transpose(0, 2, 3, 1, 4, 5).reshape(R, 2, 128, NG, NWIN, BS)
        o = o.reshape(R, 256, NG, NWIN, BS)
        kept = o[W : W + SEG]  # [seg_j, 256, NG, NWIN, b]
        for g in range(NG):
            fwd = g < 2
            for ws in range(NWIN):
                w = 8 * k + (g % 2) * 4 + ws
                hs = kept[:, :, g, ws, :]  # [seg_j, 256, b]
                if not fwd:
                    hs = hs[::-1]
                ts = np.arange(SEG * w, SEG * (w + 1))
                col0 = 0 if fwd else 256
                OUT[:, 2 * ts + l, col0 : col0 + 256] = hs.transpose(2, 0, 1)
    return OUT


def kernel(x, Wx, Wh, bx, bh):
    T = x.shape[1]
    nc = _get_nc(T)
    in_maps = prep_in_maps(x, Wx, Wh, bx, bh)
    res = run_bass_kernel_spmd(nc, in_maps, core_ids=list(range(8)))
    kernel.last_results = res
    return assemble_out([r["out"] for r in res.results], T)


# revision 6
# speedup vs baseline: 1.0775x; 1.0365x over previous
"""Bidirectional 2-layer GRU (BS=32, T=2048, D=H=256) on 8 trn2 NeuronCores.

v2: P=32 windows of SEG=64 steps (+W=12 warmup), NG=4 chains x LAN=128 lanes
(4 windows x 32 batch). Bigger tiles amortize fixed instruction overheads vs
the v1 LAN=64 design; engines balanced:
  PE   : rz-bias seed (K=4 indicator matmul) + 12 Wx + 12 Wh per group-round.
         z-gate rows of Wx/Wh (and bias) are negated on host so sigmoid
         yields z' = 1-z directly.
  DVE  : rn = (ps_hn + bhn)*r and an = (ps_xn + bxn) + rn as
         scalar_tensor_tensor pairs (per-partition bias APs fold the n-side
         biases for free); d = n - h.
  ACT  : sigmoid(ps_rz) -> [r, z'], tanh(an) -> n.
  POOL : e = d*z', h' = h + e (scalar_tensor_tensor forms, 0.6 eff).
PSUM: 4 tiles of [128, 8, 128] fp32 (2 banks each) = all 8 banks, bufs=1.
Regions: 0:4 rz (seeded with biases), 4:6 hn, 6:8 xn.
"""

import os
from contextlib import ExitStack

import numpy as np

import concourse.bass as bass
from concourse import mybir
from concourse.alu_op_type import AluOpType
from concourse.tile import TileContext
from concourse.bass_utils import run_bass_kernel_spmd

BS, T_FULL, D = 32, 2048, 256
H, L = 256, 2
NG = 4            # chains per core
LAN = 128         # lanes per chain (4 windows x 32 batch)
NWIN = 4          # windows per chain
PW = 32           # windows per direction
SEG = T_FULL // PW  # 64 steps per window
W = 12            # warmup rounds
R = SEG + W       # 76 rounds
C = 4             # rounds per DMA chunk
NCH = R // C      # 19

F16 = mybir.dt.float16
F32 = mybir.dt.float32
AF = mybir.ActivationFunctionType
AL = AluOpType


def _fix_drain_waits(nc, max_waits=1):
    """Walrus rejects instructions with >1 sync-wait: split extras onto
    single-wait NOPs just before, on the same engine."""
    n_new = 0
    for f in nc.m.functions:
        for bb in f.blocks:
            insts = list(bb.instructions)
            out = []
            changed = False
            for inst in insts:
                si = inst.sync_info
                if si and len(si.on_wait) > max_waits:
                    waits = list(si.on_wait)
                    for k, w in enumerate(waits[:-max_waits]):
                        nd = mybir.InstNoOp(name=f"{inst.name}-w{k}", ins=[], outs=[])
                        nd.engine = inst.engine
                        nd.sync_info = mybir.SyncInfo(on_wait=[w], on_update=[])
                        out.append(nd)
                        nc.register_instruction(nd, overwrite=True)
                        n_new += 1
                    inst.sync_info = mybir.SyncInfo(
                        on_wait=waits[-max_waits:], on_update=list(si.on_update)
                    )
                    changed = True
                out.append(inst)
            if changed:
                lst = bb.instructions
                lst.clear()
                lst.extend(out)
                assert [i.name for i in bb.instructions] == [i.name for i in out]
    return n_new


def _build():
    nc = bass.Bass(name="bidir_gru_v2", trn_type="TRN2")

    # x in consumption order: [ch, p, kx, j, g, lane]
    xt = nc.dram_tensor("xt", [NCH, 128, 2, C, NG, LAN], F16, kind="ExternalInput")
    # weights [p(K-half), kc*6+mt, gate-col] with z-rows negated
    wht = nc.dram_tensor("wht", [128, 12, 128], F16, kind="ExternalInput")
    wxt = nc.dram_tensor("wxt", [128, 12, 128], F16, kind="ExternalInput")
    # K=4 rz-bias seed: b4[reg, p] = bias value of gate-tile reg, col p
    b4d = nc.dram_tensor("b4d", [4, 128], F16, kind="ExternalInput")
    ind4d = nc.dram_tensor("ind4d", [4, 4, LAN], F16, kind="ExternalInput")
    # n-side biases as per-partition scalar columns
    bhn2d = nc.dram_tensor("bhn2d", [128, 2], F32, kind="ExternalInput")
    bxn2d = nc.dram_tensor("bxn2d", [128, 2], F32, kind="ExternalInput")
    hmask = nc.dram_tensor("hmask", [128, 2, NG, LAN], F16, kind="ExternalInput")
    # out[ch, p, j, kc, g, lane]
    out = nc.dram_tensor("out", [NCH, 128, C, 2, NG, LAN], F16, kind="ExternalOutput")

    with TileContext(nc) as tc, ExitStack() as ctx:
        const = ctx.enter_context(tc.tile_pool(name="const", bufs=1))
        xtp = ctx.enter_context(tc.tile_pool(name="xtp", bufs=2))
        outp = ctx.enter_context(tc.tile_pool(name="outp", bufs=2))
        psp = ctx.enter_context(tc.tile_pool(name="psp", bufs=1, space="PSUM"))
        ew = ctx.enter_context(tc.tile_pool(name="ew", bufs=3))

        b4_sb = const.tile([4, 128], F16)
        nc.sync.dma_start(out=b4_sb, in_=b4d[:, :])
        ind4 = const.tile([4, 4, LAN], F16)
        nc.sync.dma_start(out=ind4, in_=ind4d[:, :, :])
        wxt_sb = const.tile([128, 12, 128], F16)
        nc.sync.dma_start(out=wxt_sb, in_=wxt[:, :, :])
        wht_sb = const.tile([128, 12, 128], F16)
        nc.sync.dma_start(out=wht_sb, in_=wht[:, :, :])
        bhn2 = const.tile([128, 2], F32)
        nc.sync.dma_start(out=bhn2, in_=bhn2d[:, :])
        bxn2 = const.tile([128, 2], F32)
        nc.sync.dma_start(out=bxn2, in_=bxn2d[:, :])
        hmask_sb = const.tile([128, 2, NG, LAN], F16)
        nc.sync.dma_start(out=hmask_sb, in_=hmask[:, :, :, :])
        zeros = const.tile([128, 2, NG, LAN], F16)
        nc.vector.memset(zeros, 0.0)

        h_prev = [zeros[:, :, g, :] for g in range(NG)]

        outc = None
        for ch in range(NCH):
            xt_sb = xtp.tile([128, 2, C, NG, LAN], F16, tag="xt")
            nc.sync.dma_start(out=xt_sb, in_=xt[ch])
            outc_prev = outc
            outc = outp.tile([128, C, 2, NG, LAN], F16, tag="outc")
            for j in range(C):
                r = ch * C + j
                rz_all = []
                nn_all = []
                for g in range(NG):
                    # --- PE h-independent phase: rz-bias seed + Wx ---
                    # rz tile: regions 0:4 = [r0, r1, z'0, z'1] (1 PSUM bank)
                    # nn tile: regions 0:2 = hn, 2:4 = xn (1 PSUM bank)
                    rz = psp.tile([128, 4, LAN], F32, tag=f"rz{g}")
                    nn = psp.tile([128, 4, LAN], F32, tag=f"nn{g}")
                    rz_all.append(rz)
                    nn_all.append(nn)
                    # seed rz with biases (z rows negated)
                    nc.tensor.matmul(
                        out=rz.rearrange("p a b -> p (a b)"),
                        lhsT=b4_sb[:, :],
                        rhs=ind4.rearrange("k a b -> k (a b)"),
                        start=True,
                        stop=False,
                    )
                    for mt in range(4):  # Wx_rz
                        for kx in range(2):
                            nc.tensor.matmul(
                                out=rz[:, mt, :],
                                lhsT=wxt_sb[:, kx * 6 + mt, :],
                                rhs=xt_sb[:, kx, j, g, :],
                                start=False,
                                # at r=0 the Wh_rz matmuls are skipped, so
                                # the rz accumulation group ends here
                                stop=(r == 0 and mt == 3 and kx == 1),
                            )
                    for mt in range(4, 6):  # Wx_n -> nn regions 2:4
                        for kx in range(2):
                            nc.tensor.matmul(
                                out=nn[:, mt - 2, :],
                                lhsT=wxt_sb[:, kx * 6 + mt, :],
                                rhs=xt_sb[:, kx, j, g, :],
                                start=(kx == 0),
                                stop=False,
                            )
                for g in range(NG):
                    # --- PE recurrent phase: Wh (rz accum, hn fresh) ---
                    # At r=0 the state is exactly zero: skip the rz-side Wh
                    # matmuls (they add nothing); keep Wh_n for the region
                    # start=True init.
                    rz, nn = rz_all[g], nn_all[g]
                    hp = h_prev[g]
                    for mt in range(6):
                        if r == 0 and mt < 4:
                            continue
                        for kc in range(2):
                            nc.tensor.matmul(
                                out=rz[:, mt, :] if mt < 4 else nn[:, mt - 4, :],
                                lhsT=wht_sb[:, kc * 6 + mt, :],
                                rhs=hp[:, kc, :],
                                start=(mt >= 4 and kc == 0),
                                stop=(mt == 3 and kc == 1)
                                if mt < 4
                                else (mt == 5 and kc == 1),
                            )

                sgl = [None] * NG
                rnl = [None] * NG
                anl = [None] * NG
                ntl = [None] * NG
                dl = [None] * NG

                def em_sig(g):
                    # sg = [r0, r1, z'0, z'1]
                    sg = ew.tile([128, 4, LAN], F16, tag=f"sg{g}")
                    nc.scalar.activation(
                        out=sg, in_=rz_all[g], func=AF.Sigmoid
                    )
                    sgl[g] = sg

                def em_rn(g):
                    # rn = (ps_hn + bhn) * r   (2 stt, per-partition bias)
                    rn = ew.tile([128, 2, LAN], F16, tag=f"rn{g}")
                    for kc in range(2):
                        nc.vector.scalar_tensor_tensor(
                            out=rn[:, kc, :],
                            in0=nn_all[g][:, kc, :],
                            scalar=bhn2[:, kc : kc + 1],
                            in1=sgl[g][:, kc, :],
                            op0=AL.add,
                            op1=AL.mult,
                        )
                    rnl[g] = rn

                def em_an(g):
                    # an = (ps_xn + bxn) + rn  (2 stt)
                    an = ew.tile([128, 2, LAN], F16, tag=f"an{g}")
                    for kc in range(2):
                        nc.vector.scalar_tensor_tensor(
                            out=an[:, kc, :],
                            in0=nn_all[g][:, 2 + kc, :],
                            scalar=bxn2[:, kc : kc + 1],
                            in1=rnl[g][:, kc, :],
                            op0=AL.add,
                            op1=AL.add,
                        )
                    anl[g] = an

                def em_tanh(g):
                    nt = ew.tile([128, 2, LAN], F16, tag=f"nt{g}")
                    nc.scalar.activation(out=nt, in_=anl[g], func=AF.Tanh)
                    ntl[g] = nt

                def em_d(g):
                    # d = n - h ; e = d * z'  (DVE, sbuf 2x, back-to-back)
                    d = ew.tile([128, 2, LAN], F16, tag=f"d{g}")
                    nc.vector.tensor_tensor(
                        out=d, in0=ntl[g], in1=h_prev[g], op=AL.subtract
                    )
                    e = ew.tile([128, 2, LAN], F16, tag=f"e{g}")
                    nc.vector.tensor_tensor(
                        out=e, in0=d, in1=sgl[g][:, 2:4, :], op=AL.mult
                    )
                    dl[g] = e

                def em_tail(g):
                    # h' = h + e.  g0 on POOL (its deadline is earliest and
                    # POOL latency fits); g1-g3 on DVE right after their e so
                    # late-round Wh matmuls are not gated by the POOL hop.
                    dst = outc[:, j, :, g, :]
                    eng = nc.vector if g >= 1 else nc.gpsimd
                    eng.tensor_tensor(
                        out=dst, in0=h_prev[g], in1=dl[g], op=AL.add,
                    )
                    h_prev[g] = dst

                # software-pipelined interleave across the 4 chains; DVE
                # order completes ALL rn/an (they gate next round's PE
                # writes) before the d/e tail ops
                em_sig(0)
                em_rn(0)
                em_an(0)
                em_sig(1)
                em_tanh(0)
                em_rn(1)
                em_an(1)
                em_d(0)
                em_sig(2)
                em_tail(0)
                em_tanh(1)
                em_rn(2)
                em_an(2)
                em_d(1)
                em_sig(3)
                em_tail(1)
                em_tanh(2)
                em_rn(3)
                em_an(3)
                em_d(2)
                em_tail(2)
                em_tanh(3)
                em_d(3)
                em_tail(3)

                if r == W - 1:
                    # zero post-warmup state of boundary streams
                    for g in range(NG):
                        hm = outc[:, j, :, g, :]
                        nc.vector.tensor_tensor(
                            out=hm, in0=hm, in1=hmask_sb[:, :, g, :],
                            op=AL.mult,
                        )
                        h_prev[g] = hm
            nc.sync.dma_start(out=out[ch], in_=outc)
            del outc_prev

    _fix_drain_waits(nc)
    return nc


_CACHE = {}


def _get_nc(T=T_FULL):
    assert T == T_FULL, "v2 kernel is specialized to T=2048"
    if T not in _CACHE:
        _CACHE[T] = _build()
    return _CACHE[T]


def prep_in_maps(x, Wx, Wh, bx, bh):
    x = np.asarray(x, np.float32)
    Wx = np.asarray(Wx, np.float32).copy()
    Wh = np.asarray(Wh, np.float32).copy()
    bx = np.asarray(bx, np.float32)
    bh = np.asarray(bh, np.float32)
    T = x.shape[1]
    assert T == T_FULL

    # negate z-gate rows so sigmoid gives z' = 1-z
    Wx[:, 256:512, :] *= -1.0
    Wh[:, 256:512, :] *= -1.0
    brz = (bx + bh)[:, 0:512].copy()
    brz[:, 256:512] *= -1.0

    rr = np.arange(R)
    in_maps = []
    for c in range(8):
        l, k = c // 4, c % 4
        # chains: g0: fwd w 8k+0..3; g1: fwd w 8k+4..7; g2/g3: bwd same
        tidx = np.empty((NG, NWIN, R), np.int64)
        for g in range(NG):
            fwd = g < 2
            for ws in range(NWIN):
                w = 8 * k + (g % 2) * 4 + ws
                if fwd:
                    t = SEG * w - W + rr
                else:
                    t = SEG * (w + 1) - 1 + W - rr
                tidx[g, ws] = np.clip(t, 0, T - 1)
        # gather: [b, g, ws, r, d] -> [ch, p, kx, j, g, (ws, b)]
        xg = x[:, tidx, :].astype(np.float16)  # (32, NG, NWIN, R, 256)
        xg = xg.reshape(BS, NG, NWIN, NCH, C, 2, 128)
        xt_h = np.ascontiguousarray(xg.transpose(3, 6, 5, 4, 1, 2, 0)).reshape(
            NCH, 128, 2, C, NG, LAN
        )

        wht_h = np.ascontiguousarray(
            Wh[l].reshape(6, 128, 2, 128).transpose(3, 2, 0, 1).reshape(128, 12, 128),
            np.float16,
        )
        wxt_h = np.ascontiguousarray(
            Wx[l].reshape(6, 128, 2, 128).transpose(3, 2, 0, 1).reshape(128, 12, 128),
            np.float16,
        )
        b4 = brz[l].reshape(4, 128).astype(np.float16)
        ind4_h = np.zeros((4, 4, LAN), np.float16)
        for kk in range(4):
            ind4_h[kk, kk, :] = 1.0
        bhn2_h = np.ascontiguousarray(bh[l, 512:768].reshape(2, 128).T, np.float32)
        bxn2_h = np.ascontiguousarray(bx[l, 512:768].reshape(2, 128).T, np.float32)

        hm = np.ones((128, 2, NG, LAN), np.float16)
        if k == 0:
            hm[:, :, 0, 0:32] = 0.0  # fwd window 0
        if k == 3:
            hm[:, :, 3, 96:128] = 0.0  # bwd window 31
        in_maps.append(
            {
                "xt": xt_h,
                "wht": wht_h,
                "wxt": wxt_h,
                "b4d": b4,
                "ind4d": ind4_h,
                "bhn2d": bhn2_h,
                "bxn2d": bxn2_h,
                "hmask": hm,
            }
        )
    return in_maps


def assemble_out(per_core_out, T=T_FULL):
    OUT = np.empty((BS, T * L, 2 * H), np.float32)
    for c in range(8):
        l, k = c // 4, c % 4
        o = np.asarray(per_core_out[c], np.float32).reshape(NCH, 128, C, 2, NG, LAN)
        # [ch, p, j, kc, g, lane] -> [r, kc, p, g, ws, b]
        o = o.transpose(0, 2, 3, 1, 4, 5).reshape(R, 2, 128, NG, NWIN, BS)
        o = o.reshape(R, 256, NG, NWIN, BS)
        kept = o[W : W + SEG]  # [seg_j, 256, NG, NWIN, b]
        for g in range(NG):
            fwd = g < 2
            for ws in range(NWIN):
                w = 8 * k + (g % 2) * 4 + ws
                hs = kept[:, :, g, ws, :]  # [seg_j, 256, b]
                if not fwd:
                    hs = hs[::-1]
                ts = np.arange(SEG * w, SEG * (w + 1))
                col0 = 0 if fwd else 256
                OUT[:, 2 * ts + l, col0 : col0 + 256] = hs.transpose(2, 0, 1)
    return OUT


def kernel(x, Wx, Wh, bx, bh):
    T = x.shape[1]
    nc = _get_nc(T)
    in_maps = prep_in_maps(x, Wx, Wh, bx, bh)
    res = run_bass_kernel_spmd(nc, in_maps, core_ids=list(range(8)))
    kernel.last_results = res
    return assemble_out([r["out"] for r in res.results], T)


# revision 7
# speedup vs baseline: 1.0782x; 1.0006x over previous
"""Bidirectional 2-layer GRU (BS=32, T=2048, D=H=256) on 8 trn2 NeuronCores.

v2: P=32 windows of SEG=64 steps (+W=12 warmup), NG=4 chains x LAN=128 lanes
(4 windows x 32 batch). Bigger tiles amortize fixed instruction overheads vs
the v1 LAN=64 design; engines balanced:
  PE   : rz-bias seed (K=4 indicator matmul) + 12 Wx + 12 Wh per group-round.
         z-gate rows of Wx/Wh (and bias) are negated on host so sigmoid
         yields z' = 1-z directly.
  DVE  : rn = (ps_hn + bhn)*r and an = (ps_xn + bxn) + rn as
         scalar_tensor_tensor pairs (per-partition bias APs fold the n-side
         biases for free); d = n - h.
  ACT  : sigmoid(ps_rz) -> [r, z'], tanh(an) -> n.
  POOL : e = d*z', h' = h + e (scalar_tensor_tensor forms, 0.6 eff).
PSUM: 4 tiles of [128, 8, 128] fp32 (2 banks each) = all 8 banks, bufs=1.
Regions: 0:4 rz (seeded with biases), 4:6 hn, 6:8 xn.
"""

import os
from contextlib import ExitStack

import numpy as np

import concourse.bass as bass
from concourse import mybir
from concourse.alu_op_type import AluOpType
from concourse.tile import TileContext
from concourse.bass_utils import run_bass_kernel_spmd

BS, T_FULL, D = 32, 2048, 256
H, L = 256, 2
NG = 4            # chains per core
LAN = 128         # lanes per chain (4 windows x 32 batch)
NWIN = 4          # windows per chain
PW = 32           # windows per direction
SEG = T_FULL // PW  # 64 steps per window
W = 12            # warmup rounds
R = SEG + W       # 76 rounds
C = 4             # rounds per DMA chunk
NCH = R // C      # 19

F16 = mybir.dt.float16
F32 = mybir.dt.float32
AF = mybir.ActivationFunctionType
AL = AluOpType


def _fix_drain_waits(nc, max_waits=1):
    """Walrus rejects instructions with >1 sync-wait: split extras onto
    single-wait NOPs just before, on the same engine."""
    n_new = 0
    for f in nc.m.functions:
        for bb in f.blocks:
            insts = list(bb.instructions)
            out = []
            changed = False
            for inst in insts:
                si = inst.sync_info
                if si and len(si.on_wait) > max_waits:
                    waits = list(si.on_wait)
                    for k, w in enumerate(waits[:-max_waits]):
                        nd = mybir.InstNoOp(name=f"{inst.name}-w{k}", ins=[], outs=[])
                        nd.engine = inst.engine
                        nd.sync_info = mybir.SyncInfo(on_wait=[w], on_update=[])
                        out.append(nd)
                        nc.register_instruction(nd, overwrite=True)
                        n_new += 1
                    inst.sync_info = mybir.SyncInfo(
                        on_wait=waits[-max_waits:], on_update=list(si.on_update)
                    )
                    changed = True
                out.append(inst)
            if changed:
                lst = bb.instructions
                lst.clear()
                lst.extend(out)
                assert [i.name for i in bb.instructions] == [i.name for i in out]
    return n_new


def _build():
    nc = bass.Bass(name="bidir_gru_v2", trn_type="TRN2")

    # x in consumption order: [ch, p, kx, j, g, lane]
    xt = nc.dram_tensor("xt", [NCH, 128, 2, C, NG, LAN], F16, kind="ExternalInput")
    # weights [p(K-half), kc*6+mt, gate-col] with z-rows negated
    wht = nc.dram_tensor("wht", [128, 12, 128], F16, kind="ExternalInput")
    wxt = nc.dram_tensor("wxt", [128, 12, 128], F16, kind="ExternalInput")
    # K=4 rz-bias seed: b4[reg, p] = bias value of gate-tile reg, col p
    b4d = nc.dram_tensor("b4d", [4, 128], F16, kind="ExternalInput")
    ind4d = nc.dram_tensor("ind4d", [4, 4, LAN], F16, kind="ExternalInput")
    # n-side biases as per-partition scalar columns
    bhn2d = nc.dram_tensor("bhn2d", [128, 2], F32, kind="ExternalInput")
    bxn2d = nc.dram_tensor("bxn2d", [128, 2], F32, kind="ExternalInput")
    hmask = nc.dram_tensor("hmask", [128, 2, NG, LAN], F16, kind="ExternalInput")
    # out[ch, p, j, kc, g, lane]
    out = nc.dram_tensor("out", [NCH, 128, C, 2, NG, LAN], F16, kind="ExternalOutput")

    with TileContext(nc) as tc, ExitStack() as ctx:
        const = ctx.enter_context(tc.tile_pool(name="const", bufs=1))
        xtp = ctx.enter_context(tc.tile_pool(name="xtp", bufs=2))
        outp = ctx.enter_context(tc.tile_pool(name="outp", bufs=2))
        psp = ctx.enter_context(tc.tile_pool(name="psp", bufs=1, space="PSUM"))
        ew = ctx.enter_context(tc.tile_pool(name="ew", bufs=3))

        b4_sb = const.tile([4, 128], F16)
        nc.sync.dma_start(out=b4_sb, in_=b4d[:, :])
        ind4 = const.tile([4, 4, LAN], F16)
        nc.sync.dma_start(out=ind4, in_=ind4d[:, :, :])
        wxt_sb = const.tile([128, 12, 128], F16)
        nc.sync.dma_start(out=wxt_sb, in_=wxt[:, :, :])
        wht_sb = const.tile([128, 12, 128], F16)
        nc.sync.dma_start(out=wht_sb, in_=wht[:, :, :])
        bhn2 = const.tile([128, 2], F32)
        nc.sync.dma_start(out=bhn2, in_=bhn2d[:, :])
        bxn2 = const.tile([128, 2], F32)
        nc.sync.dma_start(out=bxn2, in_=bxn2d[:, :])
        hmask_sb = const.tile([128, 2, NG, LAN], F16)
        nc.sync.dma_start(out=hmask_sb, in_=hmask[:, :, :, :])
        zeros = const.tile([128, 2, NG, LAN], F16)
        nc.vector.memset(zeros, 0.0)

        h_prev = [zeros[:, :, g, :] for g in range(NG)]

        outc = None
        for ch in range(NCH):
            xt_sb = xtp.tile([128, 2, C, NG, LAN], F16, tag="xt")
            nc.sync.dma_start(out=xt_sb, in_=xt[ch])
            outc_prev = outc
            outc = outp.tile([128, C, 2, NG, LAN], F16, tag="outc")
            for j in range(C):
                r = ch * C + j
                rz_all = []
                nn_all = []
                for g in range(NG):
                    # --- PE h-independent phase: rz-bias seed + Wx_rz ---
                    # rz tile: regions 0:4 = [r0, r1, z'0, z'1] (1 PSUM bank)
                    # nn tile: regions 0:2 = hn, 2:4 = xn (1 PSUM bank)
                    rz = psp.tile([128, 4, LAN], F32, tag=f"rz{g}")
                    nn = psp.tile([128, 4, LAN], F32, tag=f"nn{g}")
                    rz_all.append(rz)
                    nn_all.append(nn)
                    # seed rz with biases (z rows negated)
                    nc.tensor.matmul(
                        out=rz.rearrange("p a b -> p (a b)"),
                        lhsT=b4_sb[:, :],
                        rhs=ind4.rearrange("k a b -> k (a b)"),
                        start=True,
                        stop=False,
                    )
                    for mt in range(4):  # Wx_rz
                        for kx in range(2):
                            nc.tensor.matmul(
                                out=rz[:, mt, :],
                                lhsT=wxt_sb[:, kx * 6 + mt, :],
                                rhs=xt_sb[:, kx, j, g, :],
                                start=False,
                                # at r=0 the Wh_rz matmuls are skipped, so
                                # the rz accumulation group ends here
                                stop=(r == 0 and mt == 3 and kx == 1),
                            )
                for g in range(NG):
                    # Wx_n after all rz work: their WAR on the previous
                    # round's rn/an readers gets maximal slack
                    nn = nn_all[g]
                    for mt in range(4, 6):  # Wx_n -> nn regions 2:4
                        for kx in range(2):
                            nc.tensor.matmul(
                                out=nn[:, mt - 2, :],
                                lhsT=wxt_sb[:, kx * 6 + mt, :],
                                rhs=xt_sb[:, kx, j, g, :],
                                start=(kx == 0),
                                stop=False,
                            )
                for g in range(NG):
                    # --- PE recurrent phase: Wh (rz accum, hn fresh) ---
                    # At r=0 the state is exactly zero: skip the rz-side Wh
                    # matmuls (they add nothing); keep Wh_n for the region
                    # start=True init.
                    rz, nn = rz_all[g], nn_all[g]
                    hp = h_prev[g]
                    for mt in range(6):
                        if r == 0 and mt < 4:
                            continue
                        for kc in range(2):
                            nc.tensor.matmul(
                                out=rz[:, mt, :] if mt < 4 else nn[:, mt - 4, :],
                                lhsT=wht_sb[:, kc * 6 + mt, :],
                                rhs=hp[:, kc, :],
                                start=(mt >= 4 and kc == 0),
                                stop=(mt == 3 and kc == 1)
                                if mt < 4
                                else (mt == 5 and kc == 1),
                            )

                sgl = [None] * NG
                rnl = [None] * NG
                anl = [None] * NG
                ntl = [None] * NG
                dl = [None] * NG

                def em_sig(g):
                    # sg = [r0, r1, z'0, z'1]
                    sg = ew.tile([128, 4, LAN], F16, tag=f"sg{g}")
                    nc.scalar.activation(
                        out=sg, in_=rz_all[g], func=AF.Sigmoid
                    )
                    sgl[g] = sg

                def em_rn(g):
                    # rn = (ps_hn + bhn) * r   (2 stt, per-partition bias)
                    rn = ew.tile([128, 2, LAN], F16, tag=f"rn{g}")
                    for kc in range(2):
                        nc.vector.scalar_tensor_tensor(
                            out=rn[:, kc, :],
                            in0=nn_all[g][:, kc, :],
                            scalar=bhn2[:, kc : kc + 1],
                            in1=sgl[g][:, kc, :],
                            op0=AL.add,
                            op1=AL.mult,
                        )
                    rnl[g] = rn

                def em_an(g):
                    # an = (ps_xn + bxn) + rn  (2 stt)
                    an = ew.tile([128, 2, LAN], F16, tag=f"an{g}")
                    for kc in range(2):
                        nc.vector.scalar_tensor_tensor(
                            out=an[:, kc, :],
                            in0=nn_all[g][:, 2 + kc, :],
                            scalar=bxn2[:, kc : kc + 1],
                            in1=rnl[g][:, kc, :],
                            op0=AL.add,
                            op1=AL.add,
                        )
                    anl[g] = an

                def em_tanh(g):
                    nt = ew.tile([128, 2, LAN], F16, tag=f"nt{g}")
                    nc.scalar.activation(out=nt, in_=anl[g], func=AF.Tanh)
                    ntl[g] = nt

                def em_d(g):
                    # d = n - h ; e = d * z'  (DVE, sbuf 2x, back-to-back)
                    d = ew.tile([128, 2, LAN], F16, tag=f"d{g}")
                    nc.vector.tensor_tensor(
                        out=d, in0=ntl[g], in1=h_prev[g], op=AL.subtract
                    )
                    e = ew.tile([128, 2, LAN], F16, tag=f"e{g}")
                    nc.vector.tensor_tensor(
                        out=e, in0=d, in1=sgl[g][:, 2:4, :], op=AL.mult
                    )
                    dl[g] = e

                def em_tail(g):
                    # h' = h + e.  g0 on POOL (its deadline is earliest and
                    # POOL latency fits); g1-g3 on DVE right after their e so
                    # late-round Wh matmuls are not gated by the POOL hop.
                    dst = outc[:, j, :, g, :]
                    eng = nc.vector if g >= 1 else nc.gpsimd
                    eng.tensor_tensor(
                        out=dst, in0=h_prev[g], in1=dl[g], op=AL.add,
                    )
                    h_prev[g] = dst

                # software-pipelined interleave across the 4 chains; DVE
                # order completes ALL rn/an (they gate next round's PE
                # writes) before the d/e tail ops
                em_sig(0)
                em_rn(0)
                em_an(0)
                em_sig(1)
                em_tanh(0)
                em_rn(1)
                em_an(1)
                em_d(0)
                em_sig(2)
                em_tail(0)
                em_tanh(1)
                em_rn(2)
                em_an(2)
                em_d(1)
                em_sig(3)
                em_tail(1)
                em_tanh(2)
                em_rn(3)
                em_an(3)
                em_d(2)
                em_tail(2)
                em_tanh(3)
                em_d(3)
                em_tail(3)

                if r == W - 1:
                    # zero post-warmup state of boundary streams
                    for g in range(NG):
                        hm = outc[:, j, :, g, :]
                        nc.vector.tensor_tensor(
                            out=hm, in0=hm, in1=hmask_sb[:, :, g, :],
                            op=AL.mult,
                        )
                        h_prev[g] = hm
            nc.sync.dma_start(out=out[ch], in_=outc)
            del outc_prev

    _fix_drain_waits(nc)
    return nc


_CACHE = {}


def _get_nc(T=T_FULL):
    assert T == T_FULL, "v2 kernel is specialized to T=2048"
    if T not in _CACHE:
        _CACHE[T] = _build()
    return _CACHE[T]


def prep_in_maps(x, Wx, Wh, bx, bh):
    x = np.asarray(x, np.float32)
    Wx = np.asarray(Wx, np.float32).copy()
    Wh = np.asarray(Wh, np.float32).copy()
    bx = np.asarray(bx, np.float32)
    bh = np.asarray(bh, np.float32)
    T = x.shape[1]
    assert T == T_FULL

    # negate z-gate rows so sigmoid gives z' = 1-z
    Wx[:, 256:512, :] *= -1.0
    Wh[:, 256:512, :] *= -1.0
    brz = (bx + bh)[:, 0:512].copy()
    brz[:, 256:512] *= -1.0

    rr = np.arange(R)
    in_maps = []
    for c in range(8):
        l, k = c // 4, c % 4
        # chains: g0: fwd w 8k+0..3; g1: fwd w 8k+4..7; g2/g3: bwd same
        tidx = np.empty((NG, NWIN, R), np.int64)
        for g in range(NG):
            fwd = g < 2
            for ws in range(NWIN):
                w = 8 * k + (g % 2) * 4 + ws
                if fwd:
                    t = SEG * w - W + rr
                else:
                    t = SEG * (w + 1) - 1 + W - rr
                tidx[g, ws] = np.clip(t, 0, T - 1)
        # gather: [b, g, ws, r, d] -> [ch, p, kx, j, g, (ws, b)]
        xg = x[:, tidx, :].astype(np.float16)  # (32, NG, NWIN, R, 256)
        xg = xg.reshape(BS, NG, NWIN, NCH, C, 2, 128)
        xt_h = np.ascontiguousarray(xg.transpose(3, 6, 5, 4, 1, 2, 0)).reshape(
            NCH, 128, 2, C, NG, LAN
        )

        wht_h = np.ascontiguousarray(
            Wh[l].reshape(6, 128, 2, 128).transpose(3, 2, 0, 1).reshape(128, 12, 128),
            np.float16,
        )
        wxt_h = np.ascontiguousarray(
            Wx[l].reshape(6, 128, 2, 128).transpose(3, 2, 0, 1).reshape(128, 12, 128),
            np.float16,
        )
        b4 = brz[l].reshape(4, 128).astype(np.float16)
        ind4_h = np.zeros((4, 4, LAN), np.float16)
        for kk in range(4):
            ind4_h[kk, kk, :] = 1.0
        bhn2_h = np.ascontiguousarray(bh[l, 512:768].reshape(2, 128).T, np.float32)
        bxn2_h = np.ascontiguousarray(bx[l, 512:768].reshape(2, 128).T, np.float32)

        hm = np.ones((128, 2, NG, LAN), np.float16)
        if k == 0:
            hm[:, :, 0, 0:32] = 0.0  # fwd window 0
        if k == 3:
            hm[:, :, 3, 96:128] = 0.0  # bwd window 31
        in_maps.append(
            {
                "xt": xt_h,
                "wht": wht_h,
                "wxt": wxt_h,
                "b4d": b4,
                "ind4d": ind4_h,
                "bhn2d": bhn2_h,
                "bxn2d": bxn2_h,
                "hmask": hm,
            }
        )
    return in_maps


def assemble_out(per_core_out, T=T_FULL):
    OUT = np.empty((BS, T * L, 2 * H), np.float32)
    for c in range(8):
        l, k = c // 4, c % 4
        o = np.asarray(per_core_out[c], np.float32).reshape(NCH, 128, C, 2, NG, LAN)
        # [ch, p, j, kc, g, lane] -> [r, kc, p, g, ws, b]
        o = o.transpose(0, 2, 3, 1, 4, 5).reshape(R, 2, 128, NG, NWIN, BS)
        o = o.reshape(R, 256, NG, NWIN, BS)
        kept = o[W : W + SEG]  # [seg_j, 256, NG, NWIN, b]
        for g in range(NG):
            fwd = g < 2
            for ws in range(NWIN):
                w = 8 * k + (g % 2) * 4 + ws
                hs = kept[:, :, g, ws, :]  # [seg_j, 256, b]
                if not fwd:
                    hs = hs[::-1]
                ts = np.arange(SEG * w, SEG * (w + 1))
                col0 = 0 if fwd else 256
                OUT[:, 2 * ts + l, col0 : col0 + 256] = hs.transpose(2, 0, 1)
    return OUT


def kernel(x, Wx, Wh, bx, bh):
    T = x.shape[1]
    nc = _get_nc(T)
    in_maps = prep_in_maps(x, Wx, Wh, bx, bh)
    res = run_bass_kernel_spmd(nc, in_maps, core_ids=list(range(8)))
    kernel.last_results = res
    return assemble_out([r["out"] for r in res.results], T)
